# revision 1
# baseline (speedup 1.0000x reference)
"""MixedExpertLayer Trainium2 kernel.

Dense data-parallel strategy: 16384 tokens sharded 8 ways (2048/core), expert
weights replicated. All 4 expert outputs are computed for every token and the
top-2 routing is applied as per-token coefficients c_e = sum_k w_k*[idx_k==e]
computed on device, so no data-dependent gather is needed.

Per-core layout: x is passed feature-major ([H, T+3] with a 3-column causal
halo) so gate/up matmuls contract H on partitions directly. A = silu(G)*U is
produced feature-major [I, T] and fed back as lhsT of the down matmul, which
yields token-major [tok, H] output. Conv experts run feature-major via
PE diagonal-matrix matmuls (4 taps accumulated in PSUM), then are transposed
into token-major with PE transpose. The final combine uses per-partition
(per-token) scalars on ACT, accumulating in SBUF bf16.

Compute dtype bf16 (PE 1 cycle/row), PSUM accumulation fp32.
"""

import numpy as np
import ml_dtypes

import concourse.bass as bass
import concourse.mybir as mybir
import concourse.tile as tile
from concourse.bass_utils import run_bass_kernel_spmd
from concourse.masks import make_identity

B, S, H, I, KTOP, KC = 4, 4096, 1024, 2048, 2, 4
NCORES = 8
T = (B * S) // NCORES          # 2048 tokens per core
TH = T + KC - 1                # 2051 cols with halo
TCH = 512                      # token chunk (matmul N / PSUM bank)
NCHUNK = T // TCH              # 4
NTS = TCH // 128               # 4 token subtiles per chunk
HK = H // 128                  # 8 h-chunks
IK = I // 128                  # 16 i-chunks
BF16 = mybir.dt.bfloat16
F32 = mybir.dt.float32
AF = mybir.ActivationFunctionType


def legalize_waits(nc):
    """This walrus build encodes exactly one sync-wait per instruction
    (single NEURON_ISA_TPB_EVENTS slot); Tile emits up to 3 plus a multi-wait
    tail Drain. Split extra waits onto wait-only EventSemaphore carriers
    inserted immediately before the instruction (same engine, same position,
    so no reordering and no deadlock risk)."""
    f = nc.m.functions[0]
    for blk in f.blocks:
        new = []
        for ins in list(blk.instructions):
            si = ins.sync_info
            if si is not None and si.on_wait and len(si.on_wait) > 1:
                best, order = {}, []
                for w in si.on_wait:
                    k = (w.sync_type, w.id, w.wait_mode)
                    if k not in best:
                        best[k] = w
                        order.append(k)
                    elif (w.wait_value or 0) > (best[k].wait_value or 0):
                        best[k] = w
                waits = [best[k] for k in order]
                for j, w in enumerate(waits[:-1]):
                    ev = mybir.InstEventSemaphore(
                        name=f"{ins.name}-lw{j}", engine=ins.engine, ins=[], outs=[],
                    )
                    ev.sync_info = mybir.SyncInfo(on_wait=[w], on_update=[])
                    new.append(ev)
                si.on_wait = [waits[-1]]
                ins.sync_info = si
            new.append(ins)
        blk.instructions = new
    return nc


def build_nc():
    nc = bass.Bass(num_devices=NCORES)
    xf = nc.dram_tensor("xf", [H, TH], BF16, kind="ExternalInput")
    wg = nc.dram_tensor("wg", [2, H, I], BF16, kind="ExternalInput")
    wu = nc.dram_tensor("wu", [2, H, I], BF16, kind="ExternalInput")
    wd = nc.dram_tensor("wd", [2, I, H], BF16, kind="ExternalInput")
    dgh = nc.dram_tensor("dgh", [2, HK, KC, 128, 128], BF16, kind="ExternalInput")
    idxp = nc.dram_tensor("idxp", [128, T // 128, KTOP], F32, kind="ExternalInput")
    nwp = nc.dram_tensor("nwp", [128, T // 128, KTOP], F32, kind="ExternalInput")
    out = nc.dram_tensor("out", [T, H], BF16, kind="ExternalOutput")

    xf_t = xf.rearrange("(o p) t -> p o t", p=128)        # [128, HK, TH]
    wg_t = [wg[e].rearrange("(o p) m -> p o m", p=128) for e in range(2)]
    wu_t = [wu[e].rearrange("(o p) m -> p o m", p=128) for e in range(2)]
    wd_t = [wd[e].rearrange("(o p) h -> p o h", p=128) for e in range(2)]

    with tile.TileContext(nc) as tc:
        with (
            tc.tile_pool(name="singles", bufs=1) as singles,
            tc.tile_pool(name="wpool", bufs=2) as wpool,
            tc.tile_pool(name="wdpool", bufs=18) as wdpool,
            tc.tile_pool(name="sf", bufs=18) as sfpool,
            tc.tile_pool(name="tmp", bufs=4) as tmp,
            tc.tile_pool(name="oa", bufs=6) as oapool,
            tc.tile_pool(name="diag", bufs=6) as diagpool,
            tc.tile_pool(name="ps", bufs=2, space="PSUM") as ps,
            tc.tile_pool(name="pd", bufs=2, space="PSUM") as pd,
        ):
            # ---- resident state ----
            xf_sb = singles.tile([128, HK, TH], BF16)
            nc.sync.dma_start(xf_sb, xf_t)

            ident = singles.tile([128, 128], BF16)
            make_identity(nc, ident)

            idxp_sb = singles.tile([128, T // 128, KTOP], F32)
            nc.sync.dma_start(idxp_sb, idxp[:])
            nwp_sb = singles.tile([128, T // 128, KTOP], F32)
            nc.sync.dma_start(nwp_sb, nwp[:])

            # routing coefficients c_tok[p, e, n] = sum_k nw[k]*[idx[k]==e]
            c_tok = singles.tile([128, 4, T // 128], F32)
            for e in range(4):
                eq = tmp.tile([128, T // 128, KTOP], F32, tag="eq")
                nc.vector.tensor_scalar(
                    out=eq, in0=idxp_sb, scalar1=float(e), scalar2=None,
                    op0=mybir.AluOpType.is_equal,
                )
                nc.vector.tensor_mul(eq, eq, nwp_sb)
                nc.vector.tensor_reduce(
                    out=c_tok[:, e, :], in_=eq, axis=mybir.AxisListType.X,
                    op=mybir.AluOpType.add,
                )

            # conv diag matrices diag(cw[e, hc*128: , j]), built host-side
            diag_sb = singles.tile([128, 2, HK, KC, 128], BF16)
            nc.sync.dma_start(diag_sb, dgh.rearrange("e hc j p m -> p e hc j m"))

            # A buffer: silu(G)*U feature-major, one expert at a time
            a_sb = singles.tile([128, IK, TCH], BF16)

            for c in range(NCHUNK):
                tok0 = c * TCH

                # ---- conv experts (2,3): feature-major, PE diag matmuls ----
                sts = {}
                for hc in range(HK):
                    for e in range(2):
                        psc = ps.tile([128, TCH], F32, tag="pg" if e == 0 else "pu")
                        for j in range(KC):
                            nc.tensor.matmul(
                                psc, diag_sb[:, e, hc, j, :],
                                xf_sb[:, hc, tok0 + j : tok0 + j + TCH],
                                start=(j == 0), stop=(j == KC - 1),
                            )
                        st = sfpool.tile([128, TCH], BF16, tag="sf")
                        nc.scalar.activation(out=st, in_=psc, func=AF.Silu)
                        sts[(e, hc)] = st

                # ---- MLP experts (0,1) ----
                for e in range(2):
                    # gate/up -> A  (feature-major [I, TCH])
                    for ig in range(4):
                        wgt = wpool.tile([128, HK, 512], BF16, tag="wg")
                        nc.sync.dma_start(wgt, wg_t[e][:, :, ig * 512 : (ig + 1) * 512])
                        wut = wpool.tile([128, HK, 512], BF16, tag="wu")
                        nc.sync.dma_start(wut, wu_t[e][:, :, ig * 512 : (ig + 1) * 512])
                        for ii in range(4):
                            i = ig * 4 + ii
                            psg = ps.tile([128, TCH], F32, tag="pg")
                            psu = ps.tile([128, TCH], F32, tag="pu")
                            for kc in range(HK):
                                nc.tensor.matmul(
                                    psg, wgt[:, kc, ii * 128 : (ii + 1) * 128],
                                    xf_sb[:, kc, 3 + tok0 : 3 + tok0 + TCH],
                                    start=(kc == 0), stop=(kc == HK - 1),
                                )
                            for kc in range(HK):
                                nc.tensor.matmul(
                                    psu, wut[:, kc, ii * 128 : (ii + 1) * 128],
                                    xf_sb[:, kc, 3 + tok0 : 3 + tok0 + TCH],
                                    start=(kc == 0), stop=(kc == HK - 1),
                                )
                            sg = tmp.tile([128, TCH], F32, tag="sg")
                            nc.scalar.activation(out=sg, in_=psg, func=AF.Silu)
                            nc.vector.tensor_mul(a_sb[:, i, :], sg, psu)

                    # down: token-major psum, post-scale by c_e
                    wds = []
                    for kc in range(IK):
                        wdt = wdpool.tile([128, H], BF16, tag="wd")
                        nc.sync.dma_start(wdt, wd_t[e][:, kc, :])
                        wds.append(wdt)
                    for ts_ in range(NTS):
                        psd = pd.tile([128, H], F32, tag="pd")
                        for kc in range(IK):
                            lhs = a_sb[:, kc, ts_ * 128 : (ts_ + 1) * 128]
                            nc.tensor.matmul(
                                psd[:, 0:512], lhs, wds[kc][:, 0:512],
                                start=(kc == 0), stop=(kc == IK - 1),
                            )
                            nc.tensor.matmul(
                                psd[:, 512:1024], lhs, wds[kc][:, 512:1024],
                                start=(kc == 0), stop=(kc == IK - 1),
                            )
                        n = c * NTS + ts_
                        if e == 0:
                            oa = oapool.tile([128, H], BF16, tag="oa")
                            sts[("oa", ts_)] = oa
                            nc.scalar.activation(
                                out=oa, in_=psd, func=AF.Copy,
                                scale=c_tok[:, 0, n : n + 1],
                            )
                        else:
                            tm = tmp.tile([128, H], BF16, tag="tm")
                            nc.scalar.activation(
                                out=tm, in_=psd, func=AF.Copy,
                                scale=c_tok[:, 1, n : n + 1],
                            )
                            oa = sts[("oa", ts_)]
                            nc.vector.tensor_add(oa, oa, tm)

                # ---- conv transpose to token-major + combine + store ----
                for ts_ in range(NTS):
                    n = c * NTS + ts_
                    oa = sts[("oa", ts_)]
                    for hg in range(2):
                        for e in range(2):
                            pst = ps.tile([128, TCH], BF16, tag="pg" if e == 0 else "pu")
                            for hh in range(4):
                                hc = hg * 4 + hh
                                nc.tensor.transpose(
                                    pst[:, hh * 128 : (hh + 1) * 128],
                                    sts[(e, hc)][:, ts_ * 128 : (ts_ + 1) * 128],
                                    ident,
                                )
                            tm = tmp.tile([128, TCH], BF16, tag="tmc")
                            nc.scalar.activation(
                                out=tm, in_=pst, func=AF.Copy,
                                scale=c_tok[:, 2 + e, n : n + 1],
                            )
                            nc.vector.tensor_add(
                                oa[:, hg * 512 : (hg + 1) * 512],
                                oa[:, hg * 512 : (hg + 1) * 512], tm,
                            )
                    nc.sync.dma_start(out[tok0 + ts_ * 128 : tok0 + (ts_ + 1) * 128, :], oa)
    return legalize_waits(nc)


def _bf16(a):
    return np.asarray(a).astype(ml_dtypes.bfloat16)


def build_in_maps(x, top_k_indices, norm_weights, mlp_gate, mlp_up, mlp_down, conv_w):
    xflat = np.asarray(x, dtype=np.float32).reshape(B * S, H)
    idxflat = np.asarray(top_k_indices).reshape(B * S, KTOP)
    nwflat = np.asarray(norm_weights, dtype=np.float32).reshape(B * S, KTOP)

    wg = _bf16(mlp_gate)
    wu = _bf16(mlp_up)
    wd = _bf16(mlp_down)
    # diag(cw[e, hc*128+p, j]) as [2, HK, KC, 128, 128]
    cw = np.asarray(conv_w, dtype=np.float32).reshape(2, HK, 128, KC)
    dgh = np.zeros((2, HK, KC, 128, 128), dtype=np.float32)
    pp = np.arange(128)
    dgh[:, :, :, pp, pp] = cw.transpose(0, 1, 3, 2)
    dgh = _bf16(dgh)

    in_maps = []
    for i in range(NCORES):
        lo = i * T
        if i % 2 == 0:
            halo = np.zeros((KC - 1, H), dtype=np.float32)
        else:
            halo = xflat[lo - (KC - 1) : lo]
        xh = np.concatenate([halo, xflat[lo : lo + T]], axis=0)  # [T+3, H]
        xf = np.ascontiguousarray(_bf16(xh).T)                   # [H, T+3]
        idxp = np.ascontiguousarray(
            idxflat[lo : lo + T].reshape(T // 128, 128, KTOP).transpose(1, 0, 2)
        ).astype(np.float32)
        nwp = np.ascontiguousarray(
            nwflat[lo : lo + T].reshape(T // 128, 128, KTOP).transpose(1, 0, 2)
        )
        in_maps.append(
            {"xf": xf, "wg": wg, "wu": wu, "wd": wd, "dgh": dgh,
             "idxp": idxp, "nwp": nwp}
        )
    return in_maps


def assemble(results):
    out = np.concatenate(
        [np.asarray(r["out"], dtype=np.float32) for r in results], axis=0
    )
    return out.reshape(B, S, H)


def kernel(x, top_k_indices, norm_weights, mlp_gate, mlp_up, mlp_down, conv_w):
    in_maps = build_in_maps(
        x, top_k_indices, norm_weights, mlp_gate, mlp_up, mlp_down, conv_w
    )
    nc = build_nc()
    res = run_bass_kernel_spmd(nc, in_maps, core_ids=list(range(NCORES)))
    return assemble(res.results)



# revision 4
# speedup vs baseline: 2.2698x; 2.2698x over previous
"""MixedExpertLayer Trainium2 kernel, v2: host-routed top-2 MoE.

16384 tokens sharded 8 ways (T=2048/core). Routing is resolved on the host:
for each MLP expert e in {0,1} the tokens with nonzero combined weight
c_e = sum_k w_k*[idx_k==e] (~43.75% of tokens) are compacted into a gathered
feature-major input xg_e [H, C]; the device runs the SwiGLU MLP only on those
columns, scales rows by c_e on ACT, and writes compacted outputs Y_e [C, H].
The host scatter-adds Y_e back into the token stream in fp32.

Conv experts (2,3) are cheap and run densely for all tokens — but on the
Vector (e2) and Pool/GpSimd (e3) engines as per-partition-scalar
multiply-accumulate taps over the feature-major halo'd x, keeping the PE
free for MLP matmuls. Their silu runs on ACT, and the weighted combine
c2*y2 + c3*y3 runs on DVE/Pool with host-prebroadcast coefficient tiles.
The conv partial output is stored feature-major [H, T]; the host transposes
and adds.

PE work: only the routed MLP matmuls (bf16, N=512 chunks): ~1536 matmuls/core
vs 3584 in the dense baseline.
"""

import numpy as np
import ml_dtypes

import concourse.bass as bass
import concourse.mybir as mybir
import concourse.tile as tile
from concourse.bass_utils import run_bass_kernel_spmd

B, S, H, I, KTOP, KC = 4, 4096, 1024, 2048, 2, 4
NCORES = 8
T = (B * S) // NCORES          # 2048 tokens per core
TH = T + KC - 1                # 2051 cols with halo
TCH = 512                      # token chunk (matmul N / PSUM bank)
NCHUNK = T // TCH              # 4 conv chunks
HK = H // 128                  # 8 h-chunks
IK = I // 128                  # 16 i-chunks
BF16 = mybir.dt.bfloat16
F32 = mybir.dt.float32
AF = mybir.ActivationFunctionType
MUL = mybir.AluOpType.mult
ADD = mybir.AluOpType.add

# capacity (padded gathered tokens per MLP expert); set by build_in_maps
_ROUTE = {"C": 1024, "lists": None}


def legalize_waits(nc):
    """This walrus build encodes exactly one sync-wait per instruction
    (single NEURON_ISA_TPB_EVENTS slot); Tile emits up to 3 plus a multi-wait
    tail Drain. Split extra waits onto wait-only EventSemaphore carriers
    inserted immediately before the instruction (same engine, same position,
    so no reordering and no deadlock risk)."""
    f = nc.m.functions[0]
    for blk in f.blocks:
        new = []
        for ins in list(blk.instructions):
            si = ins.sync_info
            if si is not None and si.on_wait and len(si.on_wait) > 1:
                best, order = {}, []
                for w in si.on_wait:
                    k = (w.sync_type, w.id, w.wait_mode)
                    if k not in best:
                        best[k] = w
                        order.append(k)
                    elif (w.wait_value or 0) > (best[k].wait_value or 0):
                        best[k] = w
                waits = [best[k] for k in order]
                for j, w in enumerate(waits[:-1]):
                    ev = mybir.InstEventSemaphore(
                        name=f"{ins.name}-lw{j}", engine=ins.engine, ins=[], outs=[],
                    )
                    ev.sync_info = mybir.SyncInfo(on_wait=[w], on_update=[])
                    new.append(ev)
                si.on_wait = [waits[-1]]
                ins.sync_info = si
            new.append(ins)
        blk.instructions = new
    return nc


def build_nc():
    C = _ROUTE["C"]
    NCH_MLP = C // TCH         # chunks per MLP expert
    NTS = TCH // 128           # 4 token subtiles per chunk

    nc = bass.Bass(num_devices=NCORES)
    xf = nc.dram_tensor("xf", [H, TH], BF16, kind="ExternalInput")
    xg = nc.dram_tensor("xg", [2, H, C], BF16, kind="ExternalInput")
    wg = nc.dram_tensor("wg", [2, H, I], BF16, kind="ExternalInput")
    wu = nc.dram_tensor("wu", [2, H, I], BF16, kind="ExternalInput")
    wd = nc.dram_tensor("wd", [2, I, H], BF16, kind="ExternalInput")
    cwp = nc.dram_tensor("cwp", [128, 2, HK, KC], F32, kind="ExternalInput")
    cgp = nc.dram_tensor("cgp", [128, 2, C // 128], F32, kind="ExternalInput")
    c23b = nc.dram_tensor("c23b", [128, 2, T], BF16, kind="ExternalInput")
    outc = nc.dram_tensor("outc", [H, T], BF16, kind="ExternalOutput")
    y = nc.dram_tensor("y", [2, C, H], BF16, kind="ExternalOutput")

    xf_t = xf.rearrange("(o p) t -> p o t", p=128)        # [128, HK, TH]
    xg_t = [xg[e].rearrange("(o p) t -> p o t", p=128) for e in range(2)]
    wg_t = [wg[e].rearrange("(o p) m -> p o m", p=128) for e in range(2)]
    wu_t = [wu[e].rearrange("(o p) m -> p o m", p=128) for e in range(2)]
    wd_t = [wd[e].rearrange("(o p) h -> p o h", p=128) for e in range(2)]
    outc_t = outc.rearrange("(o p) t -> p o t", p=128)    # [128, HK, T]

    with tile.TileContext(nc) as tc:
        with (
            tc.tile_pool(name="singles", bufs=1) as singles,
            tc.tile_pool(name="xfp", bufs=2) as xfpool,
            tc.tile_pool(name="wpool", bufs=3) as wpool,
            tc.tile_pool(name="wdpool", bufs=18) as wdpool,
            tc.tile_pool(name="apool", bufs=2) as apool,
            tc.tile_pool(name="accp", bufs=1) as accpool,
            tc.tile_pool(name="ocp", bufs=2) as ocpool,
            tc.tile_pool(name="sgp", bufs=3) as sgpool,
            tc.tile_pool(name="yap", bufs=4) as yapool,
            tc.tile_pool(name="ps", bufs=2, space="PSUM") as ps,
            tc.tile_pool(name="pd", bufs=2, space="PSUM") as pd,
        ):
            # ---- resident state ----
            xg_sb = singles.tile([128, 2, HK, C], BF16)
            for e in range(2):
                nc.sync.dma_start(xg_sb[:, e], xg_t[e])
            cw_sb = singles.tile([128, 2, HK, KC], F32)
            nc.sync.dma_start(cw_sb, cwp[:])
            cg_sb = singles.tile([128, 2, C // 128], F32)
            nc.sync.dma_start(cg_sb, cgp[:])
            c23_sb = singles.tile([128, 2, T], BF16)
            nc.sync.dma_start(c23_sb, c23b[:])

            def conv_chunk(cc):
                """Conv experts 2,3 for token chunk cc: tap-0 mul on ACT,
                taps 1-3 fused mul-add on DVE, silu on ACT, weighted combine
                on Pool (plain tensor_tensor only - Pool has no per-partition
                scalar ops on v3)."""
                t0 = cc * TCH
                xft = xfpool.tile([128, HK, TCH + KC - 1], BF16, tag="xf")
                nc.sync.dma_start(xft, xf_t[:, :, t0 : t0 + TCH + KC - 1])
                acc = [
                    accpool.tile([128, HK, TCH], BF16, tag="acc2", name="acc2"),
                    accpool.tile([128, HK, TCH], BF16, tag="acc3", name="acc3"),
                ]
                for hk in range(HK):
                    for e in range(2):
                        nc.scalar.activation(
                            out=acc[e][:, hk, :], in_=xft[:, hk, 0:TCH],
                            func=AF.Copy, scale=cw_sb[:, e, hk, 0:1],
                        )
                    for j in range(1, KC):
                        src = xft[:, hk, j : j + TCH]
                        for e in range(2):
                            nc.vector.scalar_tensor_tensor(
                                out=acc[e][:, hk, :], in0=src,
                                scalar=cw_sb[:, e, hk, j : j + 1],
                                in1=acc[e][:, hk, :], op0=MUL, op1=ADD,
                            )
                nc.scalar.activation(out=acc[0], in_=acc[0], func=AF.Silu)
                nc.scalar.activation(out=acc[1], in_=acc[1], func=AF.Silu)
                # combine: oc = c2*silu(conv2) + c3*silu(conv3), per hk on Pool
                oc = ocpool.tile([128, HK, TCH], BF16, tag="oc")
                cb = [c23_sb[:, 0, t0 : t0 + TCH], c23_sb[:, 1, t0 : t0 + TCH]]
                for hk in range(HK):
                    for e in range(2):
                        nc.gpsimd.tensor_mul(acc[e][:, hk, :], acc[e][:, hk, :], cb[e])
                for hk in range(HK):
                    nc.gpsimd.tensor_add(oc[:, hk, :], acc[0][:, hk, :], acc[1][:, hk, :])
                nc.sync.dma_start(outc_t[:, :, t0 : t0 + TCH], oc)

            phase = 0
            for e in range(2):
                for c in range(NCH_MLP):
                    # ---- gate/up -> A (feature-major [I, TCH]) ----
                    a_sb = apool.tile([128, IK, TCH], BF16, tag="a")
                    for i in range(IK):
                        wgt = wpool.tile([128, HK, 128], BF16, tag="wg")
                        nc.sync.dma_start(
                            wgt, wg_t[e][:, :, i * 128 : (i + 1) * 128])
                        wut = wpool.tile([128, HK, 128], BF16, tag="wu")
                        nc.sync.dma_start(
                            wut, wu_t[e][:, :, i * 128 : (i + 1) * 128])
                        psg = ps.tile([128, TCH], F32, tag="pg")
                        psu = ps.tile([128, TCH], F32, tag="pu")
                        for kc in range(HK):
                            nc.tensor.matmul(
                                psg, wgt[:, kc, :],
                                xg_sb[:, e, kc, c * TCH : (c + 1) * TCH],
                                start=(kc == 0), stop=(kc == HK - 1),
                            )
                        for kc in range(HK):
                            nc.tensor.matmul(
                                psu, wut[:, kc, :],
                                xg_sb[:, e, kc, c * TCH : (c + 1) * TCH],
                                start=(kc == 0), stop=(kc == HK - 1),
                            )
                        sg = sgpool.tile([128, TCH], F32, tag="sg")
                        nc.scalar.activation(out=sg, in_=psg, func=AF.Silu)
                        nc.vector.tensor_mul(a_sb[:, i, :], sg, psu)

                    # ---- down: token-major psum, post-scale by c_e ----
                    wds = []
                    for kc in range(IK):
                        wdt = wdpool.tile([128, H], BF16, tag="wd")
                        nc.sync.dma_start(wdt, wd_t[e][:, kc, :])
                        wds.append(wdt)
                    for ts_ in range(NTS):
                        psd = pd.tile([128, H], F32, tag="pd")
                        for kc in range(IK):
                            lhs = a_sb[:, kc, ts_ * 128 : (ts_ + 1) * 128]
                            nc.tensor.matmul(
                                psd[:, 0:512], lhs, wds[kc][:, 0:512],
                                start=(kc == 0), stop=(kc == IK - 1),
                            )
                            nc.tensor.matmul(
                                psd[:, 512:1024], lhs, wds[kc][:, 512:1024],
                                start=(kc == 0), stop=(kc == IK - 1),
                            )
                        n = c * NTS + ts_
                        ya = yapool.tile([128, H], BF16, tag="ya")
                        nc.scalar.activation(
                            out=ya, in_=psd, func=AF.Copy,
                            scale=cg_sb[:, e, n : n + 1],
                        )
                        row0 = c * TCH + ts_ * 128
                        nc.sync.dma_start(y[e, row0 : row0 + 128, :], ya)

                    # interleave one conv chunk per MLP phase
                    if phase < NCHUNK:
                        conv_chunk(phase)
                    phase += 1
            # any remaining conv chunks (if C < 1024 => fewer MLP phases)
            while phase < NCHUNK:
                conv_chunk(phase)
                phase += 1
    return legalize_waits(nc)


def _bf16(a):
    return np.asarray(a).astype(ml_dtypes.bfloat16)


def build_in_maps(x, top_k_indices, norm_weights, mlp_gate, mlp_up, mlp_down, conv_w):
    xflat = np.asarray(x, dtype=np.float32).reshape(B * S, H)
    idxflat = np.asarray(top_k_indices).reshape(B * S, KTOP)
    nwflat = np.asarray(norm_weights, dtype=np.float32).reshape(B * S, KTOP)

    wgb = _bf16(mlp_gate)
    wub = _bf16(mlp_up)
    wdb = _bf16(mlp_down)
    # conv weights per partition: cwp[p, e, hc, j] = conv_w[e, hc*128+p, j]
    cw = np.asarray(conv_w, dtype=np.float32).reshape(2, HK, 128, KC)
    cwp = np.ascontiguousarray(cw.transpose(2, 0, 1, 3))  # [128, 2, HK, KC]

    # per-core routing
    cores = []
    for i in range(NCORES):
        lo = i * T
        idx = idxflat[lo : lo + T]
        nw = nwflat[lo : lo + T]
        ce = np.zeros((T, 4), dtype=np.float32)
        rows = np.arange(T)
        for k in range(KTOP):
            np.add.at(ce, (rows, idx[:, k]), nw[:, k])
        lists = [np.nonzero(ce[:, e] != 0.0)[0] for e in range(2)]
        cores.append((lo, ce, lists))

    maxn = max(len(l) for (_, _, ls) in cores for l in ls)
    C = max(TCH, ((maxn + TCH - 1) // TCH) * TCH)
    _ROUTE["C"] = C
    _ROUTE["lists"] = [ls for (_, _, ls) in cores]

    in_maps = []
    for i in range(NCORES):
        lo, ce, lists = cores[i]
        if i % 2 == 0:
            halo = np.zeros((KC - 1, H), dtype=np.float32)
        else:
            halo = xflat[lo - (KC - 1) : lo]
        xh = np.concatenate([halo, xflat[lo : lo + T]], axis=0)  # [T+3, H]
        xf = np.ascontiguousarray(_bf16(xh).T)                   # [H, T+3]

        xg = np.zeros((2, H, C), dtype=ml_dtypes.bfloat16)
        cgp = np.zeros((128, 2, C // 128), dtype=np.float32)
        for e in range(2):
            lst = lists[e]
            n = len(lst)
            xg[e, :, :n] = _bf16(xflat[lo + lst]).T
            cflat = np.zeros(C, dtype=np.float32)
            cflat[:n] = ce[lst, e]
            cgp[:, e, :] = cflat.reshape(C // 128, 128).T

        c23b = np.ascontiguousarray(
            np.broadcast_to(ce[:, 2:4].T[:, None, :], (2, 128, T))
            .transpose(1, 0, 2)
        ).astype(ml_dtypes.bfloat16)  # [128, 2, T]

        in_maps.append(
            {"xf": xf, "xg": xg, "wg": wgb, "wu": wub, "wd": wdb,
             "cwp": cwp, "cgp": cgp, "c23b": c23b}
        )
    return in_maps


def assemble(results):
    lists = _ROUTE["lists"]
    out = np.empty((NCORES, T, H), dtype=np.float32)
    for i, r in enumerate(results):
        oc = np.asarray(r["outc"], dtype=np.float32).T     # [T, H]
        yv = r["y"]
        for e in range(2):
            lst = lists[i][e]
            n = len(lst)
            oc[lst] += np.asarray(yv[e, :n], dtype=np.float32)
        out[i] = oc
    return out.reshape(B, S, H)


def kernel(x, top_k_indices, norm_weights, mlp_gate, mlp_up, mlp_down, conv_w):
    in_maps = build_in_maps(
        x, top_k_indices, norm_weights, mlp_gate, mlp_up, mlp_down, conv_w
    )
    nc = build_nc()
    res = run_bass_kernel_spmd(nc, in_maps, core_ids=list(range(NCORES)))
    return assemble(res.results)
